# revision 61
# baseline (speedup 1.0000x reference)
"""AudioVisualSyncAnalyzer Trainium2 kernel (8 NeuronCores, pure data parallel).

Algorithm per sample (row of 128):
  raw cross-correlation over 255 lags via 255-point real DFT as PE matmuls
  (correlation is bilinear, so l2-normalization folds into post-scales),
  stats (argmax/max/sum/sumsq) + phase coherence + tiny MLP.

Layout per core: 8192 samples = 16 blocks x 512 samples (4 subs x 128 partitions).
"""
import sys
import numpy as np

sys.path.insert(0, "/opt/trn_rl_repo")

import concourse.bass as bass
import concourse.bacc as bacc_mod
import concourse.tile as tile
from concourse import mybir
from concourse.bass_utils import run_bass_kernel_spmd

F32 = mybir.dt.float32
F32R = mybir.dt.float32r
U32 = mybir.dt.uint32
AF = mybir.ActivationFunctionType

L = 128          # sequence length
N = 255          # DFT length (odd -> 128 bins, no junk lag)
NB = 128         # bins 0..127
B_FULL = 65536
NCORES = 8
SPC = B_FULL // NCORES      # samples per core = 8192
BLK = 512                   # samples per block
NBLK = SPC // BLK           # 16
NSUB = 4                    # 128-sample subtiles per block
PI = float(np.pi)

_CACHED = {}


def _bf16(x):
    xi = np.asarray(x, np.float32).view(np.int32)
    add = np.int32(1 << 15); mask = np.int32(~((1 << 16) - 1))
    return ((xi + add) & mask).view(np.float32)


def _consts():
    t = np.arange(L, dtype=np.float64)[:, None]
    p = np.arange(NB, dtype=np.float64)[None, :]
    ang = 2.0 * np.pi * (t * p) / N
    Wc = np.cos(ang).astype(np.float32)           # [128 t, 128 bins] lhsT
    Wsn = np.sin(ang).astype(np.float32)
    # inverse: col k (k=0..254) = np.correlate index k, circular n = (k-127)%255
    k = np.arange(N, dtype=np.int64)
    nk = (k - 127) % N
    pp = np.arange(NB, dtype=np.float64)[:, None]
    scale = np.where(pp == 0, 1.0, 2.0) / N
    angi = 2.0 * np.pi * (pp * nk[None, :]) / N
    Ci = np.zeros((NB, 256), dtype=np.float32)
    Si = np.zeros((NB, 256), dtype=np.float32)
    Ci[:, :N] = (scale * np.cos(angi)).astype(np.float32)
    Si[:, :N] = (-scale * np.sin(angi)).astype(np.float32)
    Ci[0, 255] = 1.0   # col 255 = P_0 = sum over lags of raw corr
    ident = np.eye(128, dtype=np.float32)
    # delay offset per stat column c (c = blk*4+sub): idx window offset + 127
    off = np.empty((128, 64), dtype=np.float32)
    for c in range(64):
        off[:, c] = (c % 4) * 256 + 127
    import ml_dtypes
    Cih = _bf16(Ci); Cil = _bf16(Ci - Cih)
    Sih = _bf16(Si); Sil = _bf16(Si - Sih)
    Wch = _bf16(Wc); Wcl = _bf16(Wc - Wch)
    Wsh = _bf16(Wsn); Wsl = _bf16(Wsn - Wsh)
    bf = ml_dtypes.bfloat16
    return (Wc, Wsn, Ci, Si, ident, off, Cih.astype(bf), Cil.astype(bf),
            Sih.astype(bf), Sil.astype(bf), Wch.astype(bf), Wcl.astype(bf),
            Wsh.astype(bf), Wsl.astype(bf))


def _mlp_consts(W1, b1, W2, b2):
    # block-diagonal x4 so four 128-sample subtiles share one matmul
    W1x4 = np.zeros((24, 64), dtype=np.float32)
    W2x4 = np.zeros((64, 128), dtype=np.float32)
    b1x4 = np.zeros((64, 1), dtype=np.float32)
    b2x4 = np.zeros((128, 1), dtype=np.float32)
    for s in range(4):
        W1x4[s * 6:(s + 1) * 6, s * 16:(s + 1) * 16] = W1
        W2x4[s * 16:(s + 1) * 16, s * 32:(s + 1) * 32] = W2
        b1x4[s * 16:(s + 1) * 16, 0] = b1
        b2x4[s * 32:(s + 1) * 32, 0] = b2
    return W1x4, W2x4, b1x4, b2x4


def build_nc():
    nc = bacc_mod.Bacc("TRN2", target_bir_lowering=False)
    v_in = nc.declare_dram_parameter("video", [SPC, L], F32, isOutput=False)
    a_in = nc.declare_dram_parameter("audio", [SPC, L], F32, isOutput=False)
    wc_in = nc.declare_dram_parameter("wc", [L, NB], F32, isOutput=False)
    ws_in = nc.declare_dram_parameter("ws", [L, NB], F32, isOutput=False)
    ci_in = nc.declare_dram_parameter("ci", [NB, 256], F32, isOutput=False)
    si_in = nc.declare_dram_parameter("si", [NB, 256], F32, isOutput=False)
    BF16 = mybir.dt.bfloat16
    wch_in = nc.declare_dram_parameter("wch", [L, NB], BF16, isOutput=False)
    wcl_in = nc.declare_dram_parameter("wcl", [L, NB], BF16, isOutput=False)
    wsh_in = nc.declare_dram_parameter("wsh", [L, NB], BF16, isOutput=False)
    wsl_in = nc.declare_dram_parameter("wsl", [L, NB], BF16, isOutput=False)
    cih_in = nc.declare_dram_parameter("cih", [NB, 256], BF16, isOutput=False)
    cil_in = nc.declare_dram_parameter("cil", [NB, 256], BF16, isOutput=False)
    sih_in = nc.declare_dram_parameter("sih", [NB, 256], BF16, isOutput=False)
    sil_in = nc.declare_dram_parameter("sil", [NB, 256], BF16, isOutput=False)
    id_in = nc.declare_dram_parameter("ident", [128, 128], F32, isOutput=False)
    off_in = nc.declare_dram_parameter("off", [128, 64], F32, isOutput=False)
    w1_in = nc.declare_dram_parameter("w1x4", [24, 64], F32, isOutput=False)
    w2_in = nc.declare_dram_parameter("w2x4", [64, 128], F32, isOutput=False)
    b1_in = nc.declare_dram_parameter("b1x4", [64, 1], F32, isOutput=False)
    b2_in = nc.declare_dram_parameter("b2x4", [128, 1], F32, isOutput=False)
    out = nc.declare_dram_parameter("out", [SPC, 32], F32, isOutput=True)

    with tile.TileContext(nc) as tc:
        with tc.tile_pool(name="singles", bufs=1) as singles, \
             tc.tile_pool(name="stats", bufs=1) as stats:
            wc = singles.tile([L, NB], F32, tag="wc")
            ws = singles.tile([L, NB], F32, tag="ws")
            ci = singles.tile([NB, 256], F32, tag="ci")
            si = singles.tile([NB, 256], F32, tag="si")
            wch = singles.tile([L, NB], BF16, tag="wch")
            wcl = singles.tile([L, NB], BF16, tag="wcl")
            wsh = singles.tile([L, NB], BF16, tag="wsh")
            wsl = singles.tile([L, NB], BF16, tag="wsl")
            cih = singles.tile([NB, 256], BF16, tag="cih")
            cil = singles.tile([NB, 256], BF16, tag="cil")
            sih = singles.tile([NB, 256], BF16, tag="sih")
            sil = singles.tile([NB, 256], BF16, tag="sil")
            ident = singles.tile([128, 128], F32, tag="ident")
            offs = singles.tile([128, 64], F32, tag="offs")
            w1 = singles.tile([24, 64], F32, tag="w1")
            w2 = singles.tile([64, 128], F32, tag="w2")
            b1 = singles.tile([64, 1], F32, tag="b1")
            b2 = singles.tile([128, 1], F32, tag="b2")
            zero_b = singles.tile([128, 1], F32, tag="zero_b")
            halfpi_b = singles.tile([128, 1], F32, tag="halfpi_b")
            trash = singles.tile([128, 1], F32, tag="trash")
            ones_bf = singles.tile([128, 1], BF16, tag="ones_bf")
            nc.vector.memset(ones_bf, 1.0)
            trash2 = singles.tile([128, 1], F32, tag="trash2")
            wc_r = singles.tile([L, NB], F32R, tag="wc_r")
            ws_r = singles.tile([L, NB], F32R, tag="ws_r")
            ci_r = singles.tile([NB, 256], F32R, tag="ci_r")
            si_r = singles.tile([NB, 256], F32R, tag="si_r")
            for t_, h_ in ((wc, wc_in), (ws, ws_in), (ci, ci_in), (si, si_in),
                           (cih, cih_in), (cil, cil_in), (sih, sih_in), (sil, sil_in),
                           (wch, wch_in), (wcl, wcl_in), (wsh, wsh_in), (wsl, wsl_in),
                           (ident, id_in), (offs, off_in), (w1, w1_in),
                           (w2, w2_in), (b1, b1_in), (b2, b2_in)):
                nc.sync.dma_start(out=t_, in_=h_[:])
            wpar = singles.tile([NB, 1], F32, tag="wpar")
            nc.vector.tensor_scalar(out=wpar, in0=ident[:, 0:1],
                                    scalar1=-1.0 / 255.0, scalar2=2.0 / 255.0,
                                    op0=mybir.AluOpType.mult,
                                    op1=mybir.AluOpType.add)
            # fp32r-rounded DFT constants (ACT copy = the rounding op)
            nc.scalar.copy(out=wc_r, in_=wc)
            nc.scalar.copy(out=ws_r, in_=ws)
            nc.scalar.copy(out=ci_r, in_=ci)
            nc.scalar.copy(out=si_r, in_=si)
            # ACT-local bias constants + const touches (one wait each)
            nc.scalar.mul(out=zero_b, in_=ident[:, 0:1], mul=0.0)
            nc.scalar.activation(out=halfpi_b, in_=zero_b, func=AF.Copy,
                                 bias=PI / 2, scale=0.0)
            nc.scalar.mul(out=trash[0:64], in_=b1, mul=0.0)
            nc.scalar.mul(out=trash2, in_=b2, mul=0.0)
            # PE observes const DMAs via ldweights (no PSUM write)
            for cst in (ident, wc, ws, ci, si, w1, w2, cih, cil, sih, sil, wch, wcl, wsh, wsl):
                nc.tensor.ldweights(cst.bitcast(BF16)[:, 0:128] if cst.dtype != BF16 else cst[:, 0:128])

            # per-core stat tiles, one column per (blk, sub)
            ssq = stats.tile([128, NBLK, 8], F32, tag="ssq")
            inv_all = stats.tile([128, NBLK, 8], F32, tag="inv_all")
            cmax = stats.tile([128, NBLK, 4], F32, tag="cmax")
            s_c = stats.tile([128, NBLK, 4], F32, tag="s_c")
            ssq_c = stats.tile([128, NBLK, 4], F32, tag="ssq_c")
            mc = stats.tile([128, NBLK, 4], F32, tag="mc")
            ms = stats.tile([128, NBLK, 4], F32, tag="ms")
            idxf = stats.tile([128, NBLK, 4], F32, tag="idxf")
            s_all = stats.tile([128, NBLK, 4, 6], F32, tag="s_all")

            with tc.tile_pool(name="inp", bufs=16) as inp, \
                 tc.tile_pool(name="work", bufs=3) as work, \
                 tc.tile_pool(name="work2", bufs=3) as work2, \
                 tc.tile_pool(name="sqp", bufs=1) as sqp, \
                 tc.tile_pool(name="prs", bufs=2) as prs, \
                 tc.tile_pool(name="pst", bufs=1, space="PSUM") as pst, \
                 tc.tile_pool(name="psf", bufs=1, space="PSUM") as psf, \
                 tc.tile_pool(name="psc", bufs=1, space="PSUM") as psc:
                # ---- stats assembly helper (called mid-loop for overlap) ----
                def assemble(asm, b0, b1):
                    nb = b1 - b0
                    inv_v = inv_all[:, b0:b1, 0:4]
                    inv_a = inv_all[:, b0:b1, 4:8]
                    sh = [128, nb, 4]
                    u = asm.tile(sh, F32, tag="u")
                    u2 = asm.tile(sh, F32, tag="u2")
                    tA = asm.tile(sh, F32, tag="tA")
                    tB = asm.tile(sh, F32, tag="tB")
                    tC = asm.tile(sh, F32, tag="tC")
                    sa = s_all[:, b0:b1, :, :]
                    nc.vector.tensor_mul(u, inv_v, inv_a)
                    nc.vector.tensor_mul(u2, u, u)
                    nc.vector.tensor_mul(tA, cmax[:, b0:b1, :], u)
                    nc.vector.tensor_scalar_mul(sa[:, :, :, 1], tA, 1.0 / (1.0 + 1e-6))
                    nc.vector.tensor_mul(tA, s_c[:, b0:b1, :], u)
                    nc.vector.tensor_scalar_mul(tA, tA, 1.0 / 255.0)
                    nc.vector.tensor_mul(tB, tA, tA)
                    nc.vector.tensor_mul(tC, ssq_c[:, b0:b1, :], u2)
                    nc.vector.tensor_scalar_mul(tC, tC, 1.0 / 255.0)
                    nc.vector.tensor_sub(tC, tC, tB)
                    nc.vector.tensor_scalar_max(tC, tC, 0.0)
                    nc.scalar.activation(out=sa[:, :, :, 2], in_=tC, func=AF.Sqrt, bias=zero_b)
                    nc.vector.memset(sa[:, :, :, 3], 1.0 / (1.0 + 1e-6))
                    nc.vector.tensor_sub(tA, idxf[:, b0:b1, :],
                                         offs.rearrange("p (a b) -> p a b", b=4)[:, b0:b1, :])
                    nc.vector.tensor_scalar_mul(sa[:, :, :, 0], tA, 0.1)
                    nc.vector.tensor_scalar(out=tB.bitcast(mybir.dt.int32),
                                            in0=tA.bitcast(mybir.dt.int32),
                                            scalar1=0x7FFFFFFF, scalar2=None,
                                            op0=mybir.AluOpType.bitwise_and)
                    nc.vector.tensor_scalar(out=tB, in0=tB, scalar1=1.0, scalar2=None,
                                            op0=mybir.AluOpType.add)
                    nc.vector.reciprocal(out=sa[:, :, :, 5], in_=tB)
                    nc.vector.tensor_mul(tA, mc[:, b0:b1, :], mc[:, b0:b1, :])
                    nc.vector.tensor_mul(tB, ms[:, b0:b1, :], ms[:, b0:b1, :])
                    nc.vector.tensor_add(tC, tA, tB)
                    nc.scalar.activation(out=sa[:, :, :, 4], in_=tC, func=AF.Sqrt,
                                     bias=zero_b, scale=1.0 / (128.0 * 128.0))


                v_blks, a_blks = [], []
                nall = stats.tile([128, NBLK, 8], F32, tag="nall")

                # ===== phase 0: DMA + norms (sum of squares), per block =====
                def phase0(blk):
                    r0 = blk * BLK
                    v_blk = inp.tile([128, NSUB, L], F32, tag="v_blk")
                    a_blk = inp.tile([128, NSUB, L], F32, tag="a_blk")
                    v_blks.append(v_blk); a_blks.append(a_blk)
                    nc.sync.dma_start(
                        out=v_blk, in_=v_in[r0:r0 + BLK, :].rearrange(
                            "(sub p) t -> p sub t", p=128))
                    nc.sync.dma_start(
                        out=a_blk, in_=a_in[r0:r0 + BLK, :].rearrange(
                            "(sub p) t -> p sub t", p=128))
                    # absorb DMA ticks: DVE (tiny copies) + PE (ldweights)
                    junk = inp.tile([128, 2], F32, tag="junk")
                    nc.vector.tensor_copy(out=junk[:, 0:1], in_=v_blk[:, 0, 0:1])
                    nc.vector.tensor_copy(out=junk[:, 1:2], in_=a_blk[:, 0, 0:1])
                    nc.tensor.ldweights(v_blk.bitcast(BF16)[:, 0, 0:128])
                    nc.tensor.ldweights(a_blk.bitcast(BF16)[:, 0, 0:128])
                    sq_v = sqp.tile([128, NSUB, L], F32, tag="sq_v")
                    sq_a = sqp.tile([128, NSUB, L], F32, tag="sq_a")
                    nc.gpsimd.tensor_mul(sq_v, v_blk, v_blk)
                    nc.gpsimd.tensor_mul(sq_a, a_blk, a_blk)
                    nc.vector.reduce_sum(ssq[:, blk, 0:4], sq_v, axis=mybir.AxisListType.X)
                    nc.vector.reduce_sum(ssq[:, blk, 4:8], sq_a, axis=mybir.AxisListType.X)
                def invcalc(b0, b1):
                    nc.vector.tensor_scalar_max(ssq[:, b0:b1, :],
                                                ssq[:, b0:b1, :], 1e-24)
                    nc.scalar.activation(out=nall[:, b0:b1, :],
                                         in_=ssq[:, b0:b1, :], func=AF.Sqrt,
                                         bias=zero_b)
                    nc.vector.reciprocal(out=inv_all[:, b0:b1, :],
                                         in_=nall[:, b0:b1, :])

                # ===== phase 1: DFT correlation + stats + phase coherence =====
                def phase1(blk):
                    v_blk = v_blks[blk]; a_blk = a_blks[blk]
                    # transposes
                    vT_ps = pst.tile([128, BLK], F32, tag="vT_ps")
                    aT_ps = pst.tile([128, BLK], F32, tag="aT_ps")
                    for s in range(NSUB):
                        nc.tensor.transpose(vT_ps[:, s * 128:(s + 1) * 128], v_blk[:, s, :], ident)
                        nc.tensor.transpose(aT_ps[:, s * 128:(s + 1) * 128], a_blk[:, s, :], ident)
                    vT = work.tile([128, BLK], F32R, tag="vT")
                    aT = work.tile([128, BLK], F32R, tag="aT")
                    nc.scalar.copy(out=vT, in_=vT_ps)
                    nc.scalar.copy(out=aT, in_=aT_ps)

                    # phase coherence: d in [s,t], transpose, 2 sins, PE minis
                    v_sc = work2.tile([128, NSUB, L], F32, tag="v_sc")
                    a_sc = work2.tile([128, NSUB, L], F32, tag="a_sc")
                    d_t = work2.tile([128, NSUB, L], F32, tag="d_t")
                    for s in range(NSUB):
                        nc.gpsimd.tensor_scalar_mul(v_sc[:, s, :], v_blk[:, s, :],
                                                    inv_all[:, blk, s:s + 1])
                        nc.gpsimd.tensor_scalar_mul(a_sc[:, s, :], a_blk[:, s, :],
                                                    inv_all[:, blk, s + 4:s + 5])
                    nc.gpsimd.tensor_sub(d_t, v_sc, a_sc)
                    dT_ps = pst.tile([128, BLK], F32, tag="vT_ps")
                    for s in range(NSUB):
                        nc.tensor.transpose(dT_ps[:, s * 128:(s + 1) * 128],
                                            d_t[:, s, :], ident)
                    sin_dT = work2.tile([128, BLK], BF16, tag="sin_dT")
                    cos_dT = work2.tile([128, BLK], BF16, tag="cos_dT")
                    nc.scalar.activation(out=sin_dT, in_=dT_ps, func=AF.Sin,
                                         bias=zero_b)
                    nc.scalar.activation(out=cos_dT, in_=dT_ps, func=AF.Sin,
                                         bias=halfpi_b)
                    phmini = psc.tile([128, NSUB, 256], F32, tag="corr")
                    for s in range(NSUB):
                        sl = slice(s * 128, (s + 1) * 128)
                        nc.tensor.matmul(phmini[:, 0, s:s + 1], sin_dT[:, sl],
                                         ones_bf, start=True, stop=True)
                        nc.tensor.matmul(phmini[:, 0, 4 + s:5 + s], cos_dT[:, sl],
                                         ones_bf, start=True, stop=True)
                    nc.vector.tensor_copy(out=ms[:, blk, :], in_=phmini[:, 0, 0:4])
                    nc.vector.tensor_copy(out=mc[:, blk, :], in_=phmini[:, 0, 4:8])

                    # forward DFT
                    vc_ps = psf.tile([NB, BLK], F32, tag="vc_ps")
                    vs_ps = psf.tile([NB, BLK], F32, tag="vs_ps")
                    acas_ps = psf.tile([NB, 2, BLK], F32, tag="acas_ps")
                    nc.tensor.matmul(vc_ps, wc_r, vT, start=True, stop=True)
                    nc.tensor.matmul(vs_ps, ws_r, vT, start=True, stop=True)
                    nc.tensor.matmul(acas_ps[:, 0, :], wc_r, aT, start=True,
                                     stop=True)
                    nc.tensor.matmul(acas_ps[:, 1, :], ws_r, aT, start=True,
                                     stop=True)
                    acas = work.tile([NB, 2, BLK], F32, tag="acas")
                    vs = work.tile([NB, BLK], F32, tag="vs")
                    nc.scalar.copy(out=acas, in_=acas_ps)
                    nc.scalar.copy(out=vs, in_=vs_ps)

                    # cross-spectrum: t13 = vc*[ac,as] batched on DVE (PSUM in0)
                    t13 = work.tile([NB, 2, BLK], F32, tag="t13")
                    t2 = work.tile([NB, BLK], F32, tag="t2")
                    t4 = work.tile([NB, BLK], F32, tag="t4")
                    pr = work.tile([NB, BLK], F32R, tag="pr")
                    pi = work.tile([NB, BLK], F32R, tag="pi")
                    nc.vector.tensor_tensor(
                        out=t13, in0=vc_ps.unsqueeze(1).broadcast_to([NB, 2, BLK]),
                        in1=acas, op=mybir.AluOpType.mult)
                    nc.gpsimd.tensor_mul(t2, vs, acas[:, 1, :])
                    nc.gpsimd.tensor_mul(t4, vs, acas[:, 0, :])
                    nc.gpsimd.tensor_add(pr, t13[:, 0, :], t2)
                    nc.gpsimd.tensor_sub(pi, t13[:, 1, :], t4)

                    # inverse DFT in fp32r -> corr [128 s, 4 sub, 256]
                    corr = psc.tile([128, NSUB, 256], F32, tag="corr")
                    for s in range(NSUB):
                        sl = slice(s * 128, (s + 1) * 128)
                        nc.tensor.matmul(corr[:, s, :], pr[:, sl], ci_r, start=True, stop=False)
                        nc.tensor.matmul(corr[:, s, :], pi[:, sl], si_r, start=False, stop=True)

                    # ssq_c via Parseval: sum_k wk*(pr^2+pi^2) as PE minis
                    prsq = work2.tile([NB, BLK], F32, tag="prsq")
                    pisq = work2.tile([NB, BLK], F32, tag="pisq")
                    nc.gpsimd.tensor_mul(prsq, pr.bitcast(F32), pr.bitcast(F32))
                    nc.gpsimd.tensor_mul(pisq, pi.bitcast(F32), pi.bitcast(F32))
                    smini = psc.tile([128, NSUB, 256], F32, tag="corr")
                    for s in range(NSUB):
                        sl = slice(s * 128, (s + 1) * 128)
                        nc.tensor.matmul(smini[:, 0, s:s + 1], prsq[:, sl], wpar,
                                         start=True, stop=False)
                        nc.tensor.matmul(smini[:, 0, s:s + 1], pisq[:, sl], wpar,
                                         start=False, stop=True)
                    nc.vector.tensor_copy(out=ssq_c[:, blk, :], in_=smini[:, 0, 0:4])
                    # evacuate corr once, scan from SBUF (releases PSUM early)
                    corr_sb = work2.tile([128, NSUB, 256], F32, tag="corr_sb")
                    if blk % 3 == 0:
                        nc.scalar.copy(out=corr_sb, in_=corr)
                    else:
                        nc.vector.tensor_copy(out=corr_sb, in_=corr)
                    nc.vector.reduce_max(cmax[:, blk, :], corr_sb[:, :, 0:255],
                                         axis=mybir.AxisListType.X)
                    im8 = work2.tile([128, 8], F32, tag="im8")
                    idx8 = work2.tile([128, 8], U32, tag="idx8")
                    nc.vector.tensor_copy(out=im8[:, 0:4], in_=cmax[:, blk, :])
                    nc.vector.memset(im8[:, 4:8], -3.0e38)
                    nc.vector.max_index(idx8, im8,
                                        corr_sb.rearrange("p a b -> p (a b)"))
                    nc.vector.tensor_copy(out=idxf[:, blk, :], in_=idx8[:, 0:4])
                    nc.vector.tensor_copy(out=s_c[:, blk, :], in_=corr_sb[:, :, 255])

                    if blk == 7:
                        with tc.tile_pool(name="asmA", bufs=1) as asmA:
                            assemble(asmA, 0, 8)
                    if blk == NBLK - 1:
                        with tc.tile_pool(name="asmB", bufs=1) as asmB:
                            assemble(asmB, 8, 16)

                # software pipeline: phase0 runs 8 blocks ahead of phase1,
                # sqrt/recip in two batches so phase1 can start early
                LAG = 8
                for blk in range(LAG):
                    phase0(blk)
                invcalc(0, LAG)
                for it in range(LAG, NBLK + LAG):
                    if it < NBLK:
                        phase0(it)
                        if it == NBLK - 1:
                            invcalc(LAG, NBLK)
                    phase1(it - LAG)


            # ============ phase 5: MLP (groups of 4 blocks) ============
            with tc.tile_pool(name="mlpw", bufs=4) as mlpw, \
                 tc.tile_pool(name="psm", bufs=2, space="PSUM") as psm:
                for g in range(NBLK // 4):
                    sT_ps = psm.tile([24, BLK], F32, tag="sT_ps")
                    for t_ in range(4):
                        b = g * 4 + t_
                        nc.tensor.transpose(sT_ps[:, t_ * 128:(t_ + 1) * 128],
                                            s_all[:, b, :, :].rearrange("p a b -> p (a b)"),
                                            ident)
                    sT = mlpw.tile([24, BLK], F32, tag="sT")
                    nc.scalar.copy(out=sT, in_=sT_ps)
                    h_ps = psm.tile([64, BLK], F32, tag="h_ps")
                    nc.tensor.matmul(h_ps, w1, sT, start=True, stop=True)
                    h_sb = mlpw.tile([64, BLK], F32, tag="h_sb")
                    nc.scalar.activation(out=h_sb, in_=h_ps, func=AF.Relu, bias=b1)
                    o_ps = psm.tile([128, BLK], F32, tag="o_ps")
                    nc.tensor.matmul(o_ps, w2, h_sb, start=True, stop=True)
                    o_sb = mlpw.tile([128, BLK], F32, tag="o_sb")
                    nc.scalar.activation(out=o_sb, in_=o_ps, func=AF.Identity, bias=b2)
                    oT_ps = psm.tile([128, BLK], F32, tag="oT_ps")
                    for t_ in range(4):
                        nc.tensor.transpose(oT_ps[:, t_ * 128:(t_ + 1) * 128],
                                            o_sb[:, t_ * 128:(t_ + 1) * 128], ident)
                    oT = mlpw.tile([128, BLK], F32, tag="oT")
                    nc.vector.tensor_copy(out=oT, in_=oT_ps)
                    for t_ in range(4):
                        b = g * 4 + t_
                        nc.sync.dma_start(
                            out=out[b * BLK:(b + 1) * BLK, :].rearrange(
                                "(sub p) f -> p sub f", p=128),
                            in_=oT[:, t_ * 128:(t_ + 1) * 128].rearrange(
                                "p (sub f) -> p sub f", f=32))
    if isinstance(nc, bacc_mod.Bacc):
        nc.compile()
    return nc


def kernel(video_features, audio_features, W1, b1, W2, b2):
    video_features = np.ascontiguousarray(np.asarray(video_features, dtype=np.float32))
    audio_features = np.ascontiguousarray(np.asarray(audio_features, dtype=np.float32))
    (Wc, Wsn, Ci, Si, ident, off, Cih, Cil, Sih, Sil,
     Wch, Wcl, Wsh, Wsl) = _consts()
    W1x4, W2x4, b1x4, b2x4 = _mlp_consts(
        np.asarray(W1, np.float32), np.asarray(b1, np.float32),
        np.asarray(W2, np.float32), np.asarray(b2, np.float32))

    if "nc" not in _CACHED:
        _CACHED["nc"] = build_nc()
    nc = _CACHED["nc"]

    in_maps = []
    for i in range(NCORES):
        sl = slice(i * SPC, (i + 1) * SPC)
        in_maps.append({
            "video": video_features[sl], "audio": audio_features[sl],
            "wc": Wc, "ws": Wsn, "ci": Ci, "si": Si, "ident": ident, "off": off,
            "cih": Cih, "cil": Cil, "sih": Sih, "sil": Sil,
            "wch": Wch, "wcl": Wcl, "wsh": Wsh, "wsl": Wsl,
            "w1x4": W1x4, "w2x4": W2x4, "b1x4": b1x4, "b2x4": b2x4,
        })
    res = run_bass_kernel_spmd(nc, in_maps, list(range(NCORES)))
    out = np.concatenate([res.results[i]["out"] for i in range(NCORES)], axis=0)
    return out.astype(np.float32)


if __name__ == "__main__":
    rng = np.random.default_rng(0)
    B = B_FULL
    inputs = dict(
        video_features=rng.standard_normal((B, L)).astype(np.float32),
        audio_features=rng.standard_normal((B, L)).astype(np.float32),
        W1=(rng.standard_normal((6, 16)) * 0.3).astype(np.float32),
        b1=(rng.standard_normal((16,)) * 0.1).astype(np.float32),
        W2=(rng.standard_normal((16, 32)) * 0.2).astype(np.float32),
        b2=(rng.standard_normal((32,)) * 0.1).astype(np.float32),
    )
    out = kernel(**inputs)
    print("out", out.shape, out.dtype, np.abs(out).mean())

